# revision 1
# baseline (speedup 1.0000x reference)
"""Self-contained DiT forward kernel for 8 TRN2 NeuronCores.

v3 vs v2 (driven by TimelineSim: PE-sequencer + DMA-issue bound, not FLOPs):
- One weight DMA per (GEMM, 512-col m-group) spanning ALL k-tiles
  (~40 DMAs/layer instead of ~250; SP descriptor-issue was ~1.1us each).
- Full-T GEMMs: each loaded weight tile feeds both 512-token chunks, so one
  LDWEIGHTS serves 2 matmuls (halves the PE-sequencer LDW traffic).
- Attention batched per head-pair: shared exp tile, one denominator pass of
  free-512, one broadcast matmul + one normalize-mul per pair ([128,256]).
- Keeps v2's: bf16 QKV/O/mod, fp8e4m3 DoubleRow MLP (x64 scaling),
  ln/exp-based LN rstd, ACT table-set pinning.
"""
import sys
sys.path.insert(0, "/opt/trn_rl_repo")
from contextlib import ExitStack

import numpy as np
import ml_dtypes

import concourse.bass as bass
import concourse.mybir as mybir
import concourse.tile as tile
from concourse import bacc

F32 = mybir.dt.float32
F32R = mybir.dt.float32r
BF16 = mybir.dt.bfloat16
FP8 = mybir.dt.float8e4
AF = mybir.ActivationFunctionType
ALU = mybir.AluOpType
DR = mybir.MatmulPerfMode.DoubleRow

B_FULL, D_CH, H_IMG, W_IMG = 32, 64, 32, 32
PATCH = 2
LFEAT = 256
TPI = 256
DH = 64
HPK = 2
N_CORES = 8
HS, NH, NL = 1024, 16, 12
NC_CLS = 1000
WSCALE = 64.0


class Cfg:
    def __init__(self, NIMG=4, HS=1024, NH=16, NL=12):
        self.NIMG, self.HS, self.NH, self.NL = NIMG, HS, NH, NL
        self.DFF = 4 * HS
        self.T = NIMG * TPI
        self.KT = HS // 128
        self.GKT = self.DFF // 128
        self.CW = min(512, self.T)
        self.NCH = self.T // self.CW
        self.IPC = self.CW // TPI
        assert HS % 128 == 0 and self.T % self.CW == 0 and NH == HS // DH


def _patch_act_tables():
    """Force exp/ln onto the combined natural_log_exp set (see v2)."""
    import functools
    import concourse.hw_specs as hw_specs
    if getattr(hw_specs, "_dit_act_patch", False):
        return
    orig = hw_specs.get_activation_tables.__wrapped__

    @functools.cache
    def patched(module_arch):
        tabs = dict(orig(module_arch))
        exp = mybir.ActivationFunctionType.Exp
        ln = mybir.ActivationFunctionType.Ln
        for name, fns in tabs.items():
            if name != "natural_log_exp_and_others":
                tabs[name] = fns - {exp, ln}
        return tabs

    hw_specs.get_activation_tables = patched
    hw_specs._dit_act_patch = True
    import concourse.bacc as bacc_mod
    bacc_mod.get_activation_tables = patched


def build_dit(c: Cfg):
    _patch_act_tables()
    nc = bacc.Bacc("TRN2", target_bir_lowering=False, debug=False)

    dt_ = nc.dram_tensor
    tokT_d = dt_("tokT", [LFEAT, c.T], F32R, kind="ExternalInput")
    posT_d = dt_("posT", [c.HS, TPI], F32R, kind="ExternalInput")
    cactT_d = dt_("cactT", [c.HS, c.NIMG], BF16, kind="ExternalInput")
    ident_d = dt_("ident", [4, 4], F32R, kind="ExternalInput")
    ones_d = dt_("ones", [128, 512], F32R, kind="ExternalInput")
    onesb_d = dt_("ones_bf", [1, 512], BF16, kind="ExternalInput")
    onescol_d = dt_("onescol_bf", [128, 1], BF16, kind="ExternalInput")
    projw_d = dt_("proj_w", [LFEAT, c.HS], F32R, kind="ExternalInput")
    projb_d = dt_("proj_b", [c.HS], F32, kind="ExternalInput")
    modw_d = dt_("mod_w", [c.NL, c.HS, 6 * c.HS], BF16, kind="ExternalInput")
    modb_d = dt_("mod_b", [c.NL, 1, 6 * c.HS], F32R, kind="ExternalInput")
    wq_d = dt_("wq", [c.NL, c.HS, c.HS], BF16, kind="ExternalInput")
    wk_d = dt_("wk", [c.NL, c.HS, c.HS], BF16, kind="ExternalInput")
    wv_d = dt_("wv", [c.NL, c.HS, c.HS], BF16, kind="ExternalInput")
    wo_d = dt_("wo", [c.NL, c.HS, c.HS], BF16, kind="ExternalInput")
    bq_d = dt_("bq", [c.NL, c.HS], F32, kind="ExternalInput")
    bk_d = dt_("bk", [c.NL, c.HS], F32, kind="ExternalInput")
    bv_d = dt_("bv", [c.NL, 1, c.HS], BF16, kind="ExternalInput")
    bo_d = dt_("bo", [c.NL, 1, c.HS], BF16, kind="ExternalInput")
    f1w_d = dt_("f1w", [c.NL, c.HS, c.DFF], BF16, kind="ExternalInput")
    f1b_d = dt_("f1b", [c.NL, c.DFF], F32, kind="ExternalInput")
    f2w_d = dt_("f2w", [c.NL, c.DFF, c.HS], BF16, kind="ExternalInput")
    f2b_d = dt_("f2b", [c.NL, 1, c.HS], BF16, kind="ExternalInput")
    fmodw_d = dt_("fmod_w", [c.HS, 2 * c.HS], BF16, kind="ExternalInput")
    fmodb_d = dt_("fmod_b", [1, 2 * c.HS], F32R, kind="ExternalInput")
    foutw_d = dt_("fout_w", [c.HS, LFEAT], BF16, kind="ExternalInput")
    foutb_d = dt_("fout_b", [LFEAT], F32, kind="ExternalInput")
    outT_d = dt_("outT", [LFEAT, c.T], F32, kind="ExternalOutput")

    with tile.TileContext(nc) as tc, ExitStack() as ctx:
        def pool(name, bufs, **kw):
            return ctx.enter_context(tc.tile_pool(name=name, bufs=bufs, **kw))
        const = pool("const", 1)
        resid = pool("resid", c.KT)
        hxp = pool("hxp", 1)
        qkp = pool("qkp", 2)
        vop = pool("vop", 2)
        gp = pool("gp", 1)
        modp = pool("modp", 2)
        w5p = pool("w5p", 2)
        wbp = pool("wbp", 2)
        biasp = pool("biasp", 1)
        tmpp = pool("tmpp", 2)
        rowp = pool("rowp", 1)
        pexpp = pool("pexpp", 2)
        outpp = pool("outpp", 1)
        mmp = pool("mmp", 4, space="PSUM")
        apsp = pool("apsp", 2, space="PSUM")
        spsp = pool("spsp", 2, space="PSUM")

        ident = const.tile([4, 4], F32R, tag="ident")
        nc.sync.dma_start(out=ident, in_=ident_d.ap())
        ones = const.tile([128, 512], F32R, tag="ones")
        nc.sync.dma_start(out=ones, in_=ones_d.ap())
        ones_bf = const.tile([1, 512], BF16, tag="ones_bf")
        nc.sync.dma_start(out=ones_bf, in_=onesb_d.ap())
        onescol_bf = const.tile([128, 1], BF16, tag="onescol_bf")
        nc.sync.dma_start(out=onescol_bf, in_=onescol_d.ap())
        cact_sb = const.tile([128, c.KT, c.NIMG], BF16, tag="cact")
        nc.sync.dma_start(out=cact_sb,
                          in_=cactT_d.ap().rearrange("(kt p) i -> p kt i", p=128))
        pb_sb = const.tile([128, c.KT], F32, tag="pb")
        nc.sync.dma_start(out=pb_sb,
                          in_=projb_d.ap().rearrange("(kt p) -> p kt", p=128))
        fob_sb = const.tile([128, LFEAT // 128], F32, tag="fob")
        nc.sync.dma_start(out=fob_sb,
                          in_=foutb_d.ap().rearrange("(kt p) -> p kt", p=128))
        tok_sb = qkp.tile([128, LFEAT // 128, c.T], F32R, tag="qkc")
        nc.sync.dma_start(out=tok_sb,
                          in_=tokT_d.ap().rearrange("(kt p) t -> p kt t", p=128))
        pos_sb = vop.tile([128, c.KT, TPI], F32R, tag="voc")
        nc.sync.dma_start(out=pos_sb,
                          in_=posT_d.ap().rearrange("(kt p) t -> p kt t", p=128))
        eps_sb = const.tile([1, 1], F32, tag="eps")
        nc.vector.memset(eps_sb, 1e-5)

        X = [resid.tile([128, c.T], F32R, tag="X", name=f"X{ft}")
             for ft in range(c.KT)]

        def gemm_full(w_ap, rhs_fn, nk, m_tiles, evac_fn, wpool, wdt,
                      bias_row=None, ones_row=None, wtag="w"):
            """out^T[m,:] over the full T tokens; one weight DMA per 512-col
            group spanning all k-tiles; one LDW per (m,k) feeding NCH MMs."""
            for g0 in range(0, m_tiles, 4):
                gsz = min(4, m_tiles - g0)
                wc = wpool.tile([128, nk, gsz * 128], wdt, tag=wtag)
                nc.sync.dma_start(
                    out=wc,
                    in_=w_ap[0:nk * 128, g0 * 128:(g0 + gsz) * 128]
                        .rearrange("(kt p) m -> p kt m", kt=nk))
                for mi in range(gsz):
                    m = g0 + mi
                    pss = [mmp.tile([128, 512], F32, tag="mm", name=f"p{_i}")
                           for _i in range(c.NCH)]
                    for k in range(nk):
                        for cs in range(c.NCH):
                            nc.tensor.matmul(
                                pss[cs], wc[:, k, mi * 128:(mi + 1) * 128],
                                rhs_fn(k, cs), start=(k == 0),
                                stop=(k == nk - 1 and bias_row is None))
                    if bias_row is not None:
                        for cs in range(c.NCH):
                            nc.tensor.matmul(
                                pss[cs],
                                bias_row[0:1, m * 128:(m + 1) * 128],
                                ones_row[0:1, 0:512], start=False, stop=True)
                    for cs in range(c.NCH):
                        evac_fn(m, cs, pss[cs])

        def layernorm_mod(cols0, cw, ipc, modsl, out_fn):
            ps_s = spsp.tile([1, 512], F32, tag="sps")
            ps_q = spsp.tile([1, 512], F32, tag="sps")
            for ft in range(c.KT):
                xs = X[ft][:, cols0:cols0 + cw]
                sq = tmpp.tile([128, 512], F32R, tag="xsq")
                nc.scalar.activation(sq[:, :cw], xs, AF.Square)
                nc.tensor.matmul(ps_s[:, :cw], ones[:, 0:1], xs,
                                 start=(ft == 0), stop=(ft == c.KT - 1))
                nc.tensor.matmul(ps_q[:, :cw], ones[:, 0:1], sq[:, :cw],
                                 start=(ft == 0), stop=(ft == c.KT - 1))
            lnA = rowp.tile([33, 512], F32R, tag="lnA")
            lnB = rowp.tile([65, 512], F32, tag="lnB")
            nc.scalar.activation(lnA[0:1, :cw], ps_s[:, :cw], AF.Copy,
                                 scale=1.0 / c.HS)
            mu_bc = apsp.tile([128, 512], F32, tag="aps")
            nc.tensor.matmul(mu_bc[:, :cw], ones[0:1, 0:128], lnA[0:1, :cw],
                             start=True, stop=True)
            nc.scalar.activation(lnB[0:1, :cw], lnA[0:1, :cw], AF.Square)
            nc.vector.scalar_tensor_tensor(
                lnB[32:33, :cw], ps_q[:, :cw], 1.0 / c.HS, lnB[0:1, :cw],
                op0=ALU.mult, op1=ALU.subtract)
            nc.scalar.activation(lnB[64:65, :cw], lnB[32:33, :cw], AF.Ln,
                                 bias=eps_sb[0:1, 0:1])
            nc.scalar.activation(lnA[32:33, :cw], lnB[64:65, :cw], AF.Exp,
                                 scale=-0.5)
            rs_bc = apsp.tile([128, 512], F32, tag="aps")
            nc.tensor.matmul(rs_bc[:, :cw], ones[32:33, 0:128], lnA[32:33, :cw],
                             start=True, stop=True)
            for ft in range(c.KT):
                xs = X[ft][:, cols0:cols0 + cw]
                t1 = tmpp.tile([128, 512], F32, tag="t1")
                nc.vector.tensor_sub(t1[:, :cw], xs, mu_bc[:, :cw])
                for i in range(ipc):
                    sc_ap, sh_ap = modsl(ft, i)
                    t2 = tmpp.tile([128, TPI], BF16, tag="t2")
                    nc.vector.scalar_tensor_tensor(
                        t2, t1[:, i * TPI:(i + 1) * TPI], sc_ap,
                        rs_bc[:, i * TPI:(i + 1) * TPI],
                        op0=ALU.mult, op1=ALU.mult)
                    nc.scalar.activation(out_fn(ft, i), t2, AF.Identity,
                                         bias=sh_ap)

        def mod_gemm(w_ap, b_ap, nout, dest):
            for ch in range(nout // 512):
                b_sb = rowp.tile([1, 512], F32R, tag="modb")
                nc.sync.dma_start(out=b_sb,
                                  in_=b_ap[0:1, ch * 512:(ch + 1) * 512])
                wc = w5p.tile([128, c.KT, 512], BF16, tag="w")
                nc.sync.dma_start(
                    out=wc, in_=w_ap[0:c.KT * 128, ch * 512:(ch + 1) * 512]
                        .rearrange("(kt p) m -> p kt m", kt=c.KT))
                psm = spsp.tile([c.NIMG, 512], F32, tag="sps")
                for k in range(c.KT):
                    nc.tensor.matmul(psm, cact_sb[:, k, :], wc[:, k, :],
                                     start=(k == 0), stop=False)
                nc.tensor.matmul(psm, ones[0:1, 0:c.NIMG], b_sb,
                                 start=False, stop=True)
                MTP = max(c.NIMG, 4)
                mtok = rowp.tile([MTP, 512], F32, tag="mtok")
                if MTP > c.NIMG:
                    nc.vector.memset(mtok, 0.0)
                nc.scalar.activation(mtok[0:c.NIMG, :], psm, AF.Copy)
                for j in range(4):
                    pst = apsp.tile([128, MTP], F32, tag="aps")
                    nc.tensor.transpose(pst, mtok[:, j * 128:(j + 1) * 128],
                                        ident[0:MTP, 0:MTP].bitcast(F32))
                    nc.scalar.activation(dest[:, ch * 4 + j, :],
                                         pst[:, 0:c.NIMG], AF.Copy)

        # ---------------- patchify projection ----------------
        def ev_p(mt, cs, ps):
            cc = cs * c.CW
            nc.scalar.activation(X[mt][:, cc:cc + c.CW], ps, AF.Identity,
                                 bias=pb_sb[:, mt:mt + 1])
            for i in range(c.IPC):
                s0 = cc + i * TPI
                nc.vector.tensor_add(X[mt][:, s0:s0 + TPI],
                                     X[mt][:, s0:s0 + TPI], pos_sb[:, mt, :])
        gemm_full(projw_d.ap(),
                  lambda k, cs: tok_sb[:, k, cs * c.CW:(cs + 1) * c.CW],
                  LFEAT // 128, c.KT, ev_p, w5p, F32R)

        # ---------------- transformer layers ----------------
        for l in range(c.NL):
            bqT = biasp.tile([128, c.KT], F32, tag="bqT")
            nc.sync.dma_start(out=bqT,
                              in_=bq_d.ap()[l].rearrange("(kt p) -> p kt", p=128))
            bkT = biasp.tile([128, c.KT], F32, tag="bkT")
            nc.sync.dma_start(out=bkT,
                              in_=bk_d.ap()[l].rearrange("(kt p) -> p kt", p=128))
            bv_sb = biasp.tile([1, c.HS], BF16, tag="bv")
            nc.sync.dma_start(out=bv_sb, in_=bv_d.ap()[l])
            bo_sb = biasp.tile([1, c.HS], BF16, tag="bo")
            nc.sync.dma_start(out=bo_sb, in_=bo_d.ap()[l])
            f1bT = biasp.tile([128, c.GKT], F32, tag="f1bT")
            nc.sync.dma_start(out=f1bT,
                              in_=f1b_d.ap()[l].rearrange("(kt p) -> p kt", p=128))
            f2b_sb = biasp.tile([1, c.HS], BF16, tag="f2b")
            nc.sync.dma_start(out=f2b_sb, in_=f2b_d.ap()[l])

            modT = modp.tile([128, 6 * c.KT, c.NIMG], F32, tag="modT")
            mod_gemm(modw_d.ap()[l], modb_d.ap()[l], 6 * c.HS, modT)
            nc.vector.tensor_scalar_add(modT[:, c.KT:2 * c.KT, :],
                                        modT[:, c.KT:2 * c.KT, :], 1.0)
            nc.vector.tensor_scalar_add(modT[:, 4 * c.KT:5 * c.KT, :],
                                        modT[:, 4 * c.KT:5 * c.KT, :], 1.0)

            # LN1 over both chunks -> full-T hx1 (bf16)
            hx1 = hxp.tile([128, c.KT, c.T], BF16, tag="hx")
            for chn in range(c.NCH):
                cc = chn * c.CW
                def msl_a(ft, i, _chn=chn, _m=modT):
                    gi = _chn * c.IPC + i
                    return (_m[:, c.KT + ft, gi:gi + 1], _m[:, ft, gi:gi + 1])
                layernorm_mod(cc, c.CW, c.IPC, msl_a,
                              lambda ft, i, _cc=cc:
                              hx1[:, ft, _cc + i * TPI:_cc + (i + 1) * TPI])

            Qc = qkp.tile([128, c.KT, c.T], BF16, tag="qkc")
            Kc = qkp.tile([128, c.KT, c.T], BF16, tag="qkc")
            for (w_ap, dst, bT) in ((wq_d.ap()[l], Qc, bqT),
                                    (wk_d.ap()[l], Kc, bkT)):
                def ev_qk(mt, cs, ps, _dst=dst, _bT=bT):
                    nc.vector.tensor_scalar_add(
                        _dst[:, mt, cs * c.CW:(cs + 1) * c.CW], ps,
                        _bT[:, mt:mt + 1])
                gemm_full(w_ap,
                          lambda k, cs, _h=hx1:
                          _h[:, k, cs * c.CW:(cs + 1) * c.CW],
                          c.KT, c.KT, c.CW and ev_qk, w5p, BF16)

            # V token-major, full T; stationary = token tile of hx1, so the
            # two feature-halves (fo) share each LDW.
            Vc = vop.tile([128, c.T // 128, c.HS], BF16, tag="voc")
            wcs = []
            for fo in range(2):
                wcv = w5p.tile([128, c.KT, 512], BF16, tag="w")
                nc.sync.dma_start(
                    out=wcv, in_=wv_d.ap()[l][0:c.HS, fo * 512:(fo + 1) * 512]
                        .rearrange("(kt p) m -> p kt m", kt=c.KT))
                wcs.append(wcv)
            for tt in range(c.T // 128):
                ps0 = mmp.tile([128, 512], F32, tag="mm", name="pv0")
                ps1 = mmp.tile([128, 512], F32, tag="mm", name="pv1")
                for k in range(c.KT):
                    nc.tensor.matmul(ps0, hx1[:, k, tt * 128:(tt + 1) * 128],
                                     wcs[0][:, k, :], start=(k == 0), stop=False)
                    nc.tensor.matmul(ps1, hx1[:, k, tt * 128:(tt + 1) * 128],
                                     wcs[1][:, k, :], start=(k == 0), stop=False)
                for fo, ps in ((0, ps0), (1, ps1)):
                    nc.tensor.matmul(ps, ones_bf[0:1, 0:128],
                                     bv_sb[0:1, fo * 512:(fo + 1) * 512],
                                     start=False, stop=True)
                    nc.vector.tensor_copy(Vc[:, tt, fo * 512:(fo + 1) * 512], ps)

            # attention, batched per head-pair (pair spans full 128 partitions)
            Oc = vop.tile([128, c.KT, c.T], BF16, tag="voc")
            for i in range(c.NIMG):
                i0 = i * TPI
                for j in range(c.NH // 2):
                    P2 = pexpp.tile([128, 2, 2, TPI], BF16, tag="pexp")
                    for hh in range(2):
                        r0 = DH * hh
                        s_ps = mmp.tile([128, 2, TPI], F32, tag="mm")
                        for tk in range(2):
                            nc.tensor.matmul(
                                s_ps[:, tk, :],
                                Kc[r0:r0 + DH, j,
                                   i0 + tk * 128:i0 + (tk + 1) * 128],
                                Qc[r0:r0 + DH, j, i0:i0 + TPI],
                                start=True, stop=True)
                        nc.scalar.activation(P2[:, hh], s_ps, AF.Exp,
                                             scale=0.125)
                    d_ps = spsp.tile([1, 2, TPI], F32, tag="sps")
                    for tk in range(2):
                        nc.tensor.matmul(d_ps, onescol_bf, P2[:, :, tk, :],
                                         start=(tk == 0), stop=(tk == 1))
                    rec2 = rowp.tile([1, 2, TPI], BF16, tag="rec")
                    with nc.allow_low_precision(reason="f32r storage is fp32"):
                        nc.vector.reciprocal(rec2, d_ps)
                    u_ps = mmp.tile([128, TPI], F32, tag="mm")
                    for hh in range(2):
                        h = 2 * j + hh
                        for tk in range(2):
                            nc.tensor.matmul(
                                u_ps[hh * DH:(hh + 1) * DH, :],
                                Vc[:, i * 2 + tk, h * DH:(h + 1) * DH],
                                P2[:, hh, tk, :],
                                start=(tk == 0), stop=(tk == 1))
                    bc_ps = apsp.tile([128, TPI], F32, tag="aps")
                    for hh in range(2):
                        nc.tensor.matmul(bc_ps[hh * DH:(hh + 1) * DH, :],
                                         ones_bf[0:1, 0:DH], rec2[0:1, hh, :],
                                         start=True, stop=True)
                    bc_sb = tmpp.tile([128, TPI], BF16, tag="t2")
                    nc.vector.tensor_copy(bc_sb, bc_ps)
                    nc.vector.tensor_mul(Oc[:, j, i0:i0 + TPI], u_ps, bc_sb)

            def ev_o(mt, cs, ps, _m=modT):
                for i in range(c.IPC):
                    gi = cs * c.IPC + i
                    cc = cs * c.CW
                    xa = X[mt][:, cc + i * TPI:cc + (i + 1) * TPI]
                    nc.vector.scalar_tensor_tensor(
                        xa, ps[:, i * TPI:(i + 1) * TPI],
                        _m[:, 2 * c.KT + mt, gi:gi + 1], xa,
                        op0=ALU.mult, op1=ALU.add)
            gemm_full(wo_d.ap()[l],
                      lambda k, cs, _o=Oc: _o[:, k, cs * c.CW:(cs + 1) * c.CW],
                      c.KT, c.KT, ev_o, w5p, BF16,
                      bias_row=bo_sb, ones_row=ones_bf)

            # LN2 -> full-T hx2 (fp8)
            hx2 = hxp.tile([128, c.KT, c.T], BF16, tag="hx")
            for chn in range(c.NCH):
                cc = chn * c.CW
                def msl_m(ft, i, _chn=chn, _m=modT):
                    gi = _chn * c.IPC + i
                    return (_m[:, 4 * c.KT + ft, gi:gi + 1],
                            _m[:, 3 * c.KT + ft, gi:gi + 1])
                layernorm_mod(cc, c.CW, c.IPC, msl_m,
                              lambda ft, i, _cc=cc:
                              hx2[:, ft, _cc + i * TPI:_cc + (i + 1) * TPI])

            # MLP in bf16, chunked over tokens (g too big for full-T bf16)
            def ev_m(mt, cs, ps, _m=modT):
                for i in range(c.IPC):
                    gi = cs * c.IPC + i
                    cc = cs * c.CW
                    xa = X[mt][:, cc + i * TPI:cc + (i + 1) * TPI]
                    nc.vector.scalar_tensor_tensor(
                        xa, ps[:, i * TPI:(i + 1) * TPI],
                        _m[:, 5 * c.KT + mt, gi:gi + 1], xa,
                        op0=ALU.mult, op1=ALU.add)
            for cs in range(c.NCH):
                cc = cs * c.CW
                g = gp.tile([128, c.GKT, c.CW], BF16, tag="g")
                for g0 in range(0, c.GKT, 2):
                    wc = wbp.tile([128, c.KT, 256], BF16, tag="wb")
                    nc.sync.dma_start(
                        out=wc,
                        in_=f1w_d.ap()[l][0:c.HS, g0 * 128:(g0 + 2) * 128]
                            .rearrange("(kt p) m -> p kt m", kt=c.KT))
                    pss = [mmp.tile([128, 512], F32, tag="mm", name=f"m{_i}")
                           for _i in range(2)]
                    for k in range(c.KT):
                        for mi in range(2):
                            nc.tensor.matmul(
                                pss[mi], wc[:, k, mi * 128:(mi + 1) * 128],
                                hx2[:, k, cc:cc + c.CW],
                                start=(k == 0), stop=(k == c.KT - 1))
                    for mi in range(2):
                        nc.scalar.activation(g[:, g0 + mi, :], pss[mi],
                                             AF.Gelu,
                                             bias=f1bT[:, g0 + mi:g0 + mi + 1])
                for g0 in range(0, c.KT, 2):
                    pss = [mmp.tile([128, 512], F32, tag="mm", name=f"n{_i}")
                           for _i in range(2)]
                    for kh in range(4):
                        wch = wbp.tile([128, c.GKT // 4, 256], BF16, tag="wb")
                        nc.sync.dma_start(
                            out=wch,
                            in_=f2w_d.ap()[l][kh * 1024:(kh + 1) * 1024,
                                              g0 * 128:(g0 + 2) * 128]
                                .rearrange("(kt p) m -> p kt m", kt=c.GKT // 4))
                        for krel in range(c.GKT // 4):
                            k = kh * (c.GKT // 4) + krel
                            for mi in range(2):
                                nc.tensor.matmul(
                                    pss[mi],
                                    wch[:, krel, mi * 128:(mi + 1) * 128],
                                    g[:, k, :],
                                    start=(k == 0), stop=False)
                    for mi in range(2):
                        nc.tensor.matmul(
                            pss[mi],
                            f2b_sb[0:1, (g0 + mi) * 128:(g0 + mi + 1) * 128],
                            ones_bf[0:1, 0:512], start=False, stop=True)
                        ev_m(g0 + mi, cs, pss[mi])

        # ---------------- final layer ----------------
        fmodT = modp.tile([128, 2 * c.KT, c.NIMG], F32, tag="modT")
        mod_gemm(fmodw_d.ap(), fmodb_d.ap(), 2 * c.HS, fmodT)
        nc.vector.tensor_scalar_add(fmodT[:, c.KT:2 * c.KT, :],
                                    fmodT[:, c.KT:2 * c.KT, :], 1.0)
        hxf = hxp.tile([128, c.KT, c.T], BF16, tag="hx")
        for chn in range(c.NCH):
            cc = chn * c.CW
            def msl_f(ft, i, _chn=chn, _m=fmodT):
                gi = _chn * c.IPC + i
                return (_m[:, c.KT + ft, gi:gi + 1], _m[:, ft, gi:gi + 1])
            layernorm_mod(cc, c.CW, c.IPC, msl_f,
                          lambda ft, i, _cc=cc:
                          hxf[:, ft, _cc + i * TPI:_cc + (i + 1) * TPI])
        def ev_f(mt, cs, ps):
            oc = outpp.tile([128, c.CW], F32, tag="oc")
            nc.scalar.activation(oc, ps, AF.Identity, bias=fob_sb[:, mt:mt + 1])
            nc.sync.dma_start(
                out=outT_d.ap()[mt * 128:(mt + 1) * 128,
                                cs * c.CW:(cs + 1) * c.CW], in_=oc)
        gemm_full(foutw_d.ap(),
                  lambda k, cs, _h=hxf: _h[:, k, cs * c.CW:(cs + 1) * c.CW],
                  c.KT, LFEAT // 128, ev_f, w5p, BF16)

    nc.compile()
    return nc


# ---------------- host-side pre/post-processing ----------------

def _timestep_cond(t, y, emb_table):
    half = HS // 2
    freqs = np.exp(-np.log(np.float32(10000.0)) *
                   np.arange(half, dtype=np.float32) / np.float32(half))
    ang = t.astype(np.float32)[:, None] * freqs[None]
    cemb = np.concatenate([np.cos(ang), np.sin(ang)], axis=-1)
    cc = cemb + np.asarray(emb_table, np.float32)[np.asarray(y).astype(np.int64)]
    return (cc / (1.0 + np.exp(-cc))).astype(np.float32)


def _patchify(x):
    Bc = x.shape[0]
    Hp, Wp = H_IMG // PATCH, W_IMG // PATCH
    return np.ascontiguousarray(
        x.reshape(Bc, D_CH, Hp, PATCH, Wp, PATCH)
         .transpose(0, 2, 4, 3, 5, 1).reshape(Bc, Hp * Wp, LFEAT))


def _unpatchify(tokens):
    Bc = tokens.shape[0]
    Hp, Wp = H_IMG // PATCH, W_IMG // PATCH
    return np.ascontiguousarray(
        tokens.reshape(Bc, Hp, Wp, PATCH, PATCH, D_CH)
              .transpose(0, 5, 1, 3, 2, 4).reshape(Bc, D_CH, H_IMG, W_IMG))


_CACHE = {}


def _get_nc():
    if "nc" not in _CACHE:
        _CACHE["nc"] = build_dit(Cfg(NIMG=B_FULL // N_CORES, HS=HS, NH=NH, NL=NL))
    return _CACHE["nc"]


def _fp8(w):
    w = np.asarray(w, np.float32) * WSCALE
    return np.clip(w, -240.0, 240.0).astype(ml_dtypes.float8_e4m3)


def make_shared(inputs):
    bf = ml_dtypes.bfloat16
    f = np.float32
    return {
        'posT': np.ascontiguousarray(np.asarray(inputs['pos_embed'], f).T),
        'ident': np.eye(4, dtype=f),
        'ones': np.ones((128, 512), f),
        'ones_bf': np.ones((1, 512), bf),
        'onescol_bf': np.ones((128, 1), bf),
        'proj_w': np.asarray(inputs['proj_w'], f),
        'proj_b': np.asarray(inputs['proj_b'], f),
        'mod_w': np.asarray(inputs['blk_mod_w'], f).astype(bf),
        'mod_b': np.asarray(inputs['blk_mod_b'], f)[:, None, :],
        'wq': np.asarray(inputs['blk_wq'], f).astype(bf),
        'wk': np.asarray(inputs['blk_wk'], f).astype(bf),
        'wv': np.asarray(inputs['blk_wv'], f).astype(bf),
        'wo': np.asarray(inputs['blk_wo'], f).astype(bf),
        'bq': np.asarray(inputs['blk_bq'], f),
        'bk': np.asarray(inputs['blk_bk'], f),
        'bv': np.asarray(inputs['blk_bv'], f).astype(bf)[:, None, :],
        'bo': np.asarray(inputs['blk_bo'], f).astype(bf)[:, None, :],
        'f1w': np.asarray(inputs['blk_fc1_w'], f).astype(bf),
        'f1b': np.asarray(inputs['blk_fc1_b'], f),
        'f2w': np.asarray(inputs['blk_fc2_w'], f).astype(bf),
        'f2b': np.asarray(inputs['blk_fc2_b'], f).astype(bf)[:, None, :],
        'fmod_w': np.asarray(inputs['fin_mod_w'], f).astype(bf),
        'fmod_b': np.asarray(inputs['fin_mod_b'], f)[None],
        'fout_w': np.asarray(inputs['fin_out_w'], f).astype(bf),
        'fout_b': np.asarray(inputs['fin_out_b'], f),
    }


def make_in_maps(inputs):
    f = np.float32
    bf = ml_dtypes.bfloat16
    nimg = B_FULL // N_CORES
    tok = _patchify(np.asarray(inputs['x'], f))
    c_act = _timestep_cond(np.asarray(inputs['t']), np.asarray(inputs['y']),
                           inputs['emb_table'])
    shared = make_shared(inputs)
    in_maps = []
    for cid in range(N_CORES):
        sl = slice(cid * nimg, (cid + 1) * nimg)
        im = dict(shared)
        im['tokT'] = np.ascontiguousarray(
            tok[sl].reshape(nimg * TPI, LFEAT).T)
        im['cactT'] = np.ascontiguousarray(c_act[sl].T).astype(bf)
        in_maps.append(im)
    return in_maps


def kernel(x, y, t, proj_w, proj_b, pos_embed, emb_table,
           blk_mod_w, blk_mod_b, blk_wq, blk_bq, blk_wk, blk_bk, blk_wv, blk_bv,
           blk_wo, blk_bo, blk_fc1_w, blk_fc1_b, blk_fc2_w, blk_fc2_b,
           fin_mod_w, fin_mod_b, fin_out_w, fin_out_b):
    from concourse import bass_utils

    f = np.float32
    nimg = B_FULL // N_CORES
    in_maps = make_in_maps(dict(
        x=x, y=y, t=t, proj_w=proj_w, proj_b=proj_b, pos_embed=pos_embed,
        emb_table=emb_table, blk_mod_w=blk_mod_w, blk_mod_b=blk_mod_b,
        blk_wq=blk_wq, blk_bq=blk_bq, blk_wk=blk_wk, blk_bk=blk_bk,
        blk_wv=blk_wv, blk_bv=blk_bv, blk_wo=blk_wo, blk_bo=blk_bo,
        blk_fc1_w=blk_fc1_w, blk_fc1_b=blk_fc1_b, blk_fc2_w=blk_fc2_w,
        blk_fc2_b=blk_fc2_b, fin_mod_w=fin_mod_w, fin_mod_b=fin_mod_b,
        fin_out_w=fin_out_w, fin_out_b=fin_out_b))

    nc = _get_nc()
    res = bass_utils.run_bass_kernel_spmd(nc, in_maps,
                                          core_ids=list(range(N_CORES)))
    toks_out = np.concatenate(
        [res.results[cid]['outT'].T.reshape(nimg, TPI, LFEAT)
         for cid in range(N_CORES)], axis=0)
    return _unpatchify(toks_out).astype(f)

